# revision 1
# baseline (speedup 1.0000x reference)
"""Trainium2 Bass kernel for MemoryEfficientCrossAttention.

Problem (hardcoded): B=2, Q=2048, K=4096, HIDDEN=1024, HEADS=16, HEAD_DIM=64.
  out = softmax((x_q W_q)(x_k W_k)^T / sqrt(64)) (x_v W_v) W_o

Sharding over 8 NeuronCores: core = g*4 + r
  g in {0,1}: head-group (8 heads -> 512 cols of W_q/W_k/W_v)
  r in {0..3}: 1024-row block of the flattened (B*Q, H) query (batch r//2)
Each core projects q for its rows / k,v for its batch, runs attention for its
(rows x 8 heads), then the (g=0,g=1) pair AllGathers the per-head context
(1 MiB each) and both compute the full W_o product for their row block.

Layout strategy: all matmuls contract over SBUF partitions, so activations are
needed hidden-on-partitions.  fp32 can't use the xbar DMA-transpose, so each
activation row-tile is split into two bf16 planes (hi = bf16(x),
lo = bf16(x - hi)), both planes are xbar-transposed, and the transposed fp32
value is reassembled with one DVE add -- full ~fp32 fidelity at bf16 transpose
cost.  Matmuls run as float32r (full PE rate at N>=256, ~1e-4 rel err).

Scores are computed transposed (S^T[k, q] = k q^T), so exp output is already in
the PV lhsT layout; softmax denominators come from a ones-column appended to V
(PV out row 64), and 1/r is partition-broadcast for the context scale.
"""

import os
import sys
import time

import numpy as np

sys.path.insert(0, "/opt/trn_rl_repo")

import concourse.bass as bass  # noqa: E402
import concourse.mybir as mybir  # noqa: E402
import concourse.tile as tile  # noqa: E402
from concourse import bacc  # noqa: E402
from concourse.bass_utils import run_bass_kernel_spmd  # noqa: E402

F32 = mybir.dt.float32
F32R = mybir.dt.float32r
BF16 = mybir.dt.bfloat16

HID = 1024
HEADS = 16
HD = 64
B = 2
Q = 2048
KL = 4096
NCORE = 8
GC = 512          # head-group cols per core (8 heads)
QR = 1024         # query rows per core
SCALE = HD ** -0.5

_CACHED_NC = None


def _r32(ap):
    return ap


def _build():
    nc = bacc.Bacc("TRN2", target_bir_lowering=False, debug=False,
                   num_devices=NCORE)

    q_rows = nc.dram_tensor("q_rows", [QR, HID], F32, kind="ExternalInput")
    key_b = nc.dram_tensor("key_b", [KL, HID], F32, kind="ExternalInput")
    value_b = nc.dram_tensor("value_b", [KL, HID], F32, kind="ExternalInput")
    wq_s = nc.dram_tensor("wq_s", [HID, GC], F32, kind="ExternalInput")
    wk_s = nc.dram_tensor("wk_s", [HID, GC], F32, kind="ExternalInput")
    wv_s = nc.dram_tensor("wv_s", [HID, GC], F32, kind="ExternalInput")
    w_o = nc.dram_tensor("w_o", [HID, HID], F32, kind="ExternalInput")
    outT = nc.dram_tensor("outT", [HID, QR], F32, kind="ExternalOutput")
    debug = bool(int(os.environ.get("KDEBUG", "0")))
    if debug:
        dbg_qTh = nc.dram_tensor("dbg_qTh", [GC, QR], F32,
                                 kind="ExternalOutput")
        dbg_kTh = nc.dram_tensor("dbg_kTh", [GC, KL], F32,
                                 kind="ExternalOutput")
        dbg_v = nc.dram_tensor("dbg_v", [KL, GC], F32, kind="ExternalOutput")
        dbg_ctx = nc.dram_tensor("dbg_ctx", [GC, QR], F32,
                                 kind="ExternalOutput")
        dbg_gath = nc.dram_tensor("dbg_gath", [HID, QR], F32,
                                  kind="ExternalOutput")

    precise = bool(int(os.environ.get("KPRECISE", "1")))
    NKB = KL // 128           # 32 k-blocks
    NCH = HID // 128          # 8 hidden chunks
    CH_RT = 2                 # row-tiles per transpose chunk (N=256 matmuls)
    CHR = CH_RT * 128         # 256 rows per chunk

    from contextlib import ExitStack

    with tile.TileContext(nc, pool_alloc_mode="queue") as tc:
        with tc.tile_pool(name="dram", bufs=1, space="DRAM") as dram:
            _pst = ExitStack()
            pp = _pst.enter_context(tc.tile_pool(name="persist", bufs=1))
            ctx_own = dram.tile([GC, QR], F32)
            ctx_gath = dram.tile([4, 2 * 128, QR], F32)
            qTh = pp.tile([128, 4, QR], F32R)          # [pair cols, strips, rows]
            v_aug = pp.tile([128, NKB, 8, HD + 1], F32R)
            kTh = pp.tile([128, 4, KL], F32R)

            ones = pp.tile([128, NKB * 8], F32, name="ones")
            nc.vector.memset(ones[:], 1.0)
            nc.vector.tensor_copy(
                v_aug[:, :, :, HD],
                ones[:].rearrange("p (a b) -> p a b", a=NKB))

            # ---------------- projections ----------------
            with (
                tc.tile_pool(name="xstage", bufs=1) as xs,
                tc.tile_pool(name="hilo", bufs=1) as hl,
                tc.tile_pool(name="hiloT", bufs=1) as hlt,
                tc.tile_pool(name="xT", bufs=2) as xts,
                tc.tile_pool(name="wslice", bufs=1) as ws,
                tc.tile_pool(name="pproj", bufs=4, space="PSUM") as pj,
            ):
                def stage_chunk(src, row0):
                    """Return [128, NCH, CHR] f32r transposed chunk of src."""
                    xf = xs.tile([128, CH_RT, HID], F32, tag="xf")
                    for t in range(CH_RT):
                        r0 = row0 + t * 128
                        nc.sync.dma_start(xf[:, t, :], src[r0:r0 + 128, :])
                    hi = hl.tile([128, CH_RT, HID], BF16, tag="hi")
                    nc.vector.tensor_copy(hi[:], xf[:])
                    lo = hl.tile([128, CH_RT, HID], BF16, tag="lo")
                    nc.vector.tensor_sub(lo[:], xf[:], hi[:])
                    hiT = hlt.tile([128, NCH, CHR], BF16, tag="hiT")
                    loT = hlt.tile([128, NCH, CHR], BF16, tag="loT")
                    for t in range(CH_RT):
                        sl = slice(t * 128, (t + 1) * 128)
                        nc.sync.dma_start_transpose(hiT[:, :, sl], hi[:, t, :])
                        nc.sync.dma_start_transpose(loT[:, :, sl], lo[:, t, :])
                    xT = xts.tile([128, NCH, CHR], F32R, tag="xT")
                    nc.vector.tensor_add(xT[:], hiT[:], loT[:])
                    return [xT]

                def load_w(wdram):
                    w = ws.tile([128, NCH, GC], F32R, tag="w")
                    for hc in range(NCH):
                        nc.gpsimd.dma_start(w[:, hc, :],
                                            wdram[hc * 128:(hc + 1) * 128, :])
                    return w

                # q projection -> qTh strips
                w = load_w(wq_s)
                for c in range(QR // CHR):
                    xTh = stage_chunk(q_rows, c * CHR)
                    for s in range(4):
                        ps = pj.tile([128, CHR], F32, tag="pqk")
                        for hc in range(NCH):
                            nc.tensor.matmul(
                                ps[:],
                                _r32(w[:, hc, s * 128:(s + 1) * 128]),
                                _r32(xTh[0][:, hc, :]),
                                start=(hc == 0), stop=(hc == NCH - 1))
                        sl = slice(c * CHR, (c + 1) * CHR)
                        nc.vector.tensor_copy(qTh[:, s, sl], ps[:])

                # k projection -> kTh strips staged to DRAM
                w = load_w(wk_s)
                for c in range(KL // CHR):
                    xTh = stage_chunk(key_b, c * CHR)
                    for s in range(4):
                        ps = pj.tile([128, CHR], F32, tag="pqk")
                        for hc in range(NCH):
                            nc.tensor.matmul(
                                ps[:],
                                _r32(w[:, hc, s * 128:(s + 1) * 128]),
                                _r32(xTh[0][:, hc, :]),
                                start=(hc == 0), stop=(hc == NCH - 1))
                        nc.vector.tensor_copy(
                            kTh[:, s, c * CHR:(c + 1) * CHR], ps[:])

                # v projection -> v_aug natural layout
                w = load_w(wv_s)
                for c in range(KL // CHR):
                    xTh = stage_chunk(value_b, c * CHR)
                    for t in range(CH_RT):
                        ps = pj.tile([128, GC], F32, tag="pv")
                        for hc in range(NCH):
                            nc.tensor.matmul(
                                ps[:],
                                _r32(xTh[0][:, hc,
                                         t * 128:(t + 1) * 128]),
                                _r32(w[:, hc, :]),
                                start=(hc == 0), stop=(hc == NCH - 1))
                        kb = c * CH_RT + t
                        nc.vector.tensor_copy(
                            v_aug[:, kb, :, 0:HD],
                            ps[:].rearrange("p (h d) -> p h d", h=8))

            # ---------------- attention ----------------
            GKB = 3  # k-blocks per score/exp group (3 PSUM banks)
            with (
                tc.tile_pool(name="ctxp", bufs=1) as ctxp,
                tc.tile_pool(name="apool", bufs=3) as apool,
                tc.tile_pool(name="misc", bufs=2) as misc,
                tc.tile_pool(name="pst", bufs=1, space="PSUM") as pst,
                tc.tile_pool(name="pctx", bufs=1, space="PSUM") as pctx,
            ):
                ctxall = ctxp.tile([128, 4, QR], F32)
                for s in range(4):
                    for qb in range(2):
                        qsl = slice(qb * 512, (qb + 1) * 512)
                        ctxs = [pctx.tile([HD + 1, 512], F32, tag=f"ctx{i}",
                                          name=f"ctx{i}_{s}_{qb}")
                                for i in range(2)]
                        for g0 in range(0, NKB, GKB):
                            kbs = list(range(g0, min(g0 + GKB, NKB)))
                            L = len(kbs)
                            sts = [pst.tile([128, GKB * 512], F32,
                                            tag=f"st{i}",
                                            name=f"st{i}_{s}_{qb}_{g0}")
                                   for i in range(2)]
                            for j, kb in enumerate(kbs):
                                ksl = slice(kb * 128, (kb + 1) * 128)
                                jsl = slice(j * 512, (j + 1) * 512)
                                for i in range(2):
                                    psl = slice(i * 64, (i + 1) * 64)
                                    nc.tensor.matmul(
                                        sts[i][:, jsl],
                                        _r32(kTh[psl, s, ksl]),
                                        _r32(qTh[psl, s, qsl]),
                                        start=True, stop=True)
                            As = []
                            for i in range(2):
                                a = apool.tile([128, GKB * 512], F32R,
                                               tag=f"A{i}",
                                               name=f"A{i}_{s}_{qb}_{g0}")
                                nc.scalar.activation(
                                    a[:, 0:L * 512], sts[i][:, 0:L * 512],
                                    mybir.ActivationFunctionType.Exp,
                                    scale=SCALE)
                                As.append(a)
                            for j, kb in enumerate(kbs):
                                jsl = slice(j * 512, (j + 1) * 512)
                                for i in range(2):
                                    nc.tensor.matmul(
                                        ctxs[i][:],
                                        _r32(v_aug[:, kb, 2 * s + i, :]),
                                        _r32(As[i][:, jsl]),
                                        start=(kb == 0), stop=(kb == NKB - 1))
                        for i in range(2):
                            rinv = misc.tile([1, 512], F32, tag="rinv")
                            nc.vector.reciprocal(rinv[:], ctxs[i][HD:HD + 1, :])
                            rb = misc.tile([HD, 512], F32, tag="rb")
                            nc.gpsimd.partition_broadcast(rb[:], rinv[:])
                            nc.vector.tensor_mul(
                                ctxall[i * 64:(i + 1) * 64, s, qsl],
                                ctxs[i][0:HD, :], rb[:])
                    nc.sync.dma_start(ctx_own[s * 128:(s + 1) * 128, :],
                                      ctxall[:, s, :])
                    nc.gpsimd.collective_compute(
                        "AllGather", mybir.AluOpType.bypass,
                        ins=[ctx_own[s * 128:(s + 1) * 128, :]],
                        outs=[ctx_gath[s]],
                        replica_groups=[[0, 4], [1, 5], [2, 6], [3, 7]])

            if debug:
                for s in range(4):
                    nc.gpsimd.dma_start(dbg_qTh[s * 128:(s + 1) * 128, :],
                                        qTh[:, s, :])
                for s_ in range(4):
                    nc.gpsimd.dma_start(dbg_kTh[s_ * 128:(s_ + 1) * 128, :],
                                        kTh[:, s_, :])
                for kb in range(NKB):
                    nc.gpsimd.dma_start(
                        dbg_v[kb * 128:(kb + 1) * 128, :].rearrange(
                            "p (h d) -> p h d", h=8),
                        v_aug[:, kb, :, 0:HD])

            # ---------------- pair exchange of context ----------------
            _pst.close()

            if debug:
                nc.gpsimd.dma_start(dbg_ctx[:], ctx_own[:])
                for st in range(4):
                    for g in range(2):
                        nc.gpsimd.dma_start(
                            dbg_gath[g * GC + st * 128:g * GC + (st + 1) * 128, :],
                            ctx_gath[st, g * 128:(g + 1) * 128, :])

            # ---------------- output projection ----------------
            with (
                tc.tile_pool(name="wo", bufs=1) as wop,
                tc.tile_pool(name="pwo", bufs=4, space="PSUM") as pwo,
            ):
                wo_sb = wop.tile([128, NCH, HID], F32R)
                ctxg = wop.tile([128, NCH, QR], F32R)
                outT_sb = wop.tile([128, NCH, QR], F32)
                for hc in range(NCH):
                    hsl = slice(hc * 128, (hc + 1) * 128)
                    nc.gpsimd.dma_start(wo_sb[:, hc, :], w_o[hsl, :])
                    g, st = hc // 4, hc % 4
                    nc.gpsimd.dma_start(
                        ctxg[:, hc, :],
                        ctx_gath[st, g * 128:(g + 1) * 128, :])
                for oc in range(NCH):
                    for half in range(2):
                        ps = pwo.tile([128, 512], F32, tag="po")
                        hsl = slice(half * 512, (half + 1) * 512)
                        for hc in range(NCH):
                            nc.tensor.matmul(
                                ps[:],
                                _r32(wo_sb[:, hc, oc * 128:(oc + 1) * 128]),
                                _r32(ctxg[:, hc, hsl]),
                                start=(hc == 0), stop=(hc == NCH - 1))
                        nc.vector.tensor_copy(outT_sb[:, oc, hsl], ps[:])
                for oc in range(NCH):
                    nc.sync.dma_start(outT[oc * 128:(oc + 1) * 128, :],
                                      outT_sb[:, oc, :])

    nc.compile()
    return nc


def _get_nc():
    global _CACHED_NC
    if _CACHED_NC is None:
        _CACHED_NC = _build()
    return _CACHED_NC


def make_in_maps(query, key, value, w_q, w_k, w_v, w_o):
    qf = np.ascontiguousarray(query.reshape(B * Q, HID))
    ins = []
    for core in range(NCORE):
        g, r = core // 4, core % 4
        b = r // 2
        ins.append({
            "q_rows": np.ascontiguousarray(qf[r * QR:(r + 1) * QR]),
            "key_b": np.ascontiguousarray(key[b]),
            "value_b": np.ascontiguousarray(value[b]),
            "wq_s": np.ascontiguousarray(w_q[:, g * GC:(g + 1) * GC]),
            "wk_s": np.ascontiguousarray(w_k[:, g * GC:(g + 1) * GC]),
            "wv_s": np.ascontiguousarray(w_v[:, g * GC:(g + 1) * GC]),
            "w_o": np.ascontiguousarray(w_o),
        })
    return ins


def assemble(results):
    out = np.empty((B * Q, HID), np.float32)
    for r in range(4):
        out[r * QR:(r + 1) * QR, :] = results[r]["outT"].T
    return out.reshape(B, Q, HID)


_EXEC = None


def _get_exec():
    """Build the 8-core shard_map executable once; reuse across calls."""
    global _EXEC
    if _EXEC is not None:
        return _EXEC
    import jax
    from jax.sharding import Mesh, PartitionSpec
    from jax.experimental.shard_map import shard_map
    from concourse.bass2jax import (_bass_exec_p, install_neuronx_cc_hook,
                                    partition_id_tensor)

    install_neuronx_cc_hook()
    nc = _get_nc()
    in_names, out_names, out_avals, zero_outs = [], [], [], []
    for alloc in nc.m.functions[0].allocations:
        if not isinstance(alloc, mybir.MemoryLocationSet):
            continue
        name = alloc.memorylocations[0].name
        if alloc.kind == "ExternalInput":
            if name != "partition_id":
                in_names.append(name)
        elif alloc.kind == "ExternalOutput":
            out_names.append(name)
            shape = tuple(alloc.tensor_shape)
            dtype = mybir.dt.np(alloc.dtype)
            out_avals.append(jax.core.ShapedArray(shape, dtype))
            zero_outs.append(np.zeros(shape, dtype))
    partition_name = (nc.partition_id_tensor.name
                      if nc.partition_id_tensor else None)
    all_in = list(in_names) + list(out_names)
    if partition_name:
        all_in.append(partition_name)

    def _body(*args):
        operands = list(args)
        if partition_name is not None:
            operands.append(partition_id_tensor())
        return tuple(_bass_exec_p.bind(
            *operands, out_avals=tuple(out_avals), in_names=tuple(all_in),
            out_names=tuple(out_names), lowering_input_output_aliases=(),
            sim_require_finite=True, sim_require_nnan=True, nc=nc))

    devices = jax.devices()[:NCORE]
    mesh = Mesh(np.asarray(devices), ("core",))
    n_all = len(in_names) + len(out_names)
    fn = jax.jit(shard_map(_body, mesh=mesh,
                           in_specs=(PartitionSpec("core"),) * n_all,
                           out_specs=(PartitionSpec("core"),) * len(out_names),
                           check_rep=False), keep_unused=True)
    concat_zeros = [np.zeros((NCORE * z.shape[0], *z.shape[1:]), z.dtype)
                    for z in zero_outs]
    _EXEC = (fn, in_names, out_names, out_avals, concat_zeros)
    return _EXEC


def kernel(query, key, value, w_q, w_k, w_v, w_o):
    query = np.asarray(query, dtype=np.float32)
    key = np.asarray(key, dtype=np.float32)
    value = np.asarray(value, dtype=np.float32)
    ins = make_in_maps(query, key, value, np.asarray(w_q, np.float32),
                       np.asarray(w_k, np.float32), np.asarray(w_v, np.float32),
                       np.asarray(w_o, np.float32))
    fn, in_names, out_names, out_avals, concat_zeros = _get_exec()
    concat_in = [np.concatenate([np.asarray(ins[c][nm]) for c in range(NCORE)])
                 for nm in in_names]
    out_arrs = fn(*concat_in, *concat_zeros)
    results = [
        {nm: np.asarray(out_arrs[i]).reshape(NCORE, *out_avals[i].shape)[c]
         for i, nm in enumerate(out_names)}
        for c in range(NCORE)]
    return assemble(results)


if __name__ == "__main__":
    np.random.seed(0)
    q = np.random.randn(B, Q, HID).astype(np.float32)
    k = np.random.randn(B, KL, HID).astype(np.float32)
    v = np.random.randn(B, KL, HID).astype(np.float32)
    s = 1.0 / np.sqrt(HID)
    wq = (np.random.randn(HID, HID) * s).astype(np.float32)
    wk = (np.random.randn(HID, HID) * s).astype(np.float32)
    wv = (np.random.randn(HID, HID) * s).astype(np.float32)
    wo = (np.random.randn(HID, HID) * s).astype(np.float32)
    t0 = time.time()
    out = kernel(q, k, v, wq, wk, wv, wo)
    print("kernel done", time.time() - t0, out.shape)



# revision 2
# speedup vs baseline: 2.0804x; 2.0804x over previous
"""Trainium2 Bass kernel for MemoryEfficientCrossAttention (bf16 rewrite).

Problem (hardcoded): B=2, Q=2048, K=4096, HIDDEN=1024, HEADS=16, HEAD_DIM=64.
  out = softmax((x_q W_q)(x_k W_k)^T / sqrt(64)) (x_v W_v) W_o

Sharding over 8 NeuronCores: core = g*4 + r
  g in {0,1}: head-group (8 heads -> 512 cols of W_q/W_k/W_v)
  r in {0..3}: 1024-row block of the flattened (B*Q, H) query (batch r//2)

All activations/weights are converted to bf16 and pre-transposed on the host
(hidden-major), so the device does no transposition at all: projections
contract hidden over SBUF partitions directly, scores are computed transposed
(S^T[k,q]) so exp output is already in PV lhsT layout, softmax denominators
come from a ones-column appended to V.  The (g=0,g=1) pair AllGathers the
normalized per-head context in bf16 and each core computes its own 512-column
half of the W_o product (the host stitches the halves).
"""

import os
import sys
import time

import numpy as np

sys.path.insert(0, "/opt/trn_rl_repo")

import concourse.bass as bass  # noqa: E402
import concourse.mybir as mybir  # noqa: E402
import concourse.tile as tile  # noqa: E402
from concourse import bacc  # noqa: E402

try:
    import ml_dtypes  # noqa: E402
    BF16_NP = ml_dtypes.bfloat16
except ImportError:  # jax ships ml_dtypes; fallback just in case
    import jax.numpy as jnp  # noqa: E402
    BF16_NP = jnp.bfloat16

F32 = mybir.dt.float32
BF16 = mybir.dt.bfloat16

HID = 1024
HEADS = 16
HD = 64
B = 2
Q = 2048
KL = 4096
NCORE = 8
GC = 512          # head-group cols per core (8 heads)
QR = 1024         # query rows per core
OC = 512          # out-proj cols per core (g-half of HID)
SCALE = HD ** -0.5

_CACHED_NC = None


def _build(repeat=1):
    """Build the kernel program.  repeat>1 unrolls the whole computation
    N times inside one program (reusing the same tiles, so iterations
    serialize through data deps) -- used only for HW timing, where the
    per-dispatch RPC overhead of the axon PJRT path must be amortized."""
    nc = bacc.Bacc("TRN2", target_bir_lowering=False, debug=False,
                   num_devices=NCORE)

    qT = nc.dram_tensor("qT", [HID, QR], BF16, kind="ExternalInput")
    kT = nc.dram_tensor("kT", [HID, KL], BF16, kind="ExternalInput")
    vT = nc.dram_tensor("vT", [HID, KL], BF16, kind="ExternalInput")
    wq_s = nc.dram_tensor("wq_s", [HID, GC], BF16, kind="ExternalInput")
    wk_s = nc.dram_tensor("wk_s", [HID, GC], BF16, kind="ExternalInput")
    wv_s = nc.dram_tensor("wv_s", [HID, GC], BF16, kind="ExternalInput")
    wo_s = nc.dram_tensor("wo_s", [HID, OC], BF16, kind="ExternalInput")
    outT_h = nc.dram_tensor("outT_h", [OC, QR], F32, kind="ExternalOutput")

    NKB = KL // 128           # 32 k-blocks
    NCH = HID // 128          # 8 hidden chunks
    GKB = 3                   # k-blocks per score/exp group (3 PSUM banks)

    with tile.TileContext(nc, pool_alloc_mode="queue") as tc:
        with tc.tile_pool(name="dram", bufs=1, space="DRAM") as dram:
            from contextlib import ExitStack
            _pst = ExitStack()
            pp = _pst.enter_context(tc.tile_pool(name="persist", bufs=1))
            ctx_own = dram.tile([GC, QR], BF16)
            ctx_gath = dram.tile([4, 2 * 128, QR], BF16)
            qTh = pp.tile([128, 4, QR], BF16)       # [pair cols, strip, q rows]
            kTh = pp.tile([128, 4, KL], BF16)
            v_aug = pp.tile([128, NKB, 8, HD + 1], BF16)

            ones = pp.tile([128, NKB * 8], BF16, name="ones")
            nc.vector.memset(ones[:], 1.0)
            nc.vector.tensor_copy(
                v_aug[:, :, :, HD],
                ones[:].rearrange("p (a b) -> p a b", a=NKB))

            wo_sb = pp.tile([128, NCH, OC], BF16, name="wo_sb")
            ctxg = pp.tile([128, NCH, QR], BF16, name="ctxg")
            for _rep in range(repeat):
              # ---------------- projections ----------------
            with (
                tc.tile_pool(name="xstage", bufs=2) as xs,
                tc.tile_pool(name="wslice", bufs=1) as ws,
                tc.tile_pool(name="pproj", bufs=4, space="PSUM") as pj,
            ):
                def load_xT(src, c0, cols):
                    """[128, NCH, cols] bf16 hidden-major tile of src."""
                    xt = xs.tile([128, NCH, 512], BF16, tag="xT")
                    nc.sync.dma_start(
                        xt[:, :, 0:cols],
                        src[:, c0:c0 + cols].rearrange(
                            "(hc p) q -> p hc q", p=128))
                    return xt

                def load_w(wdram, cols):
                    w = ws.tile([128, NCH, cols], BF16, tag="w")
                    nc.sync.dma_start(
                        w[:], wdram[:, 0:cols].rearrange(
                            "(hc p) c -> p hc c", p=128))
                    return w

                # q projection -> qTh strips
                w = load_w(wq_s, GC)
                for c in range(QR // 512):
                    xt = load_xT(qT, c * 512, 512)
                    for s in range(4):
                        ps = pj.tile([128, 512], F32, tag="pqk")
                        for hc in range(NCH):
                            nc.tensor.matmul(
                                ps[:],
                                w[:, hc, s * 128:(s + 1) * 128],
                                xt[:, hc, :],
                                start=(hc == 0), stop=(hc == NCH - 1))
                        nc.vector.tensor_copy(
                            qTh[:, s, c * 512:(c + 1) * 512], ps[:])

                # k projection -> kTh strips
                w = load_w(wk_s, GC)
                for c in range(KL // 512):
                    xt = load_xT(kT, c * 512, 512)
                    for s in range(4):
                        ps = pj.tile([128, 512], F32, tag="pqk")
                        for hc in range(NCH):
                            nc.tensor.matmul(
                                ps[:],
                                w[:, hc, s * 128:(s + 1) * 128],
                                xt[:, hc, :],
                                start=(hc == 0), stop=(hc == NCH - 1))
                        nc.vector.tensor_copy(
                            kTh[:, s, c * 512:(c + 1) * 512], ps[:])

                # v projection -> v_aug natural layout [k rows, heads, d]
                w = load_w(wv_s, GC)
                for c in range(KL // 512):
                    xt = load_xT(vT, c * 512, 512)
                    for t in range(4):
                        ps = pj.tile([128, GC], F32, tag="pv")
                        for hc in range(NCH):
                            nc.tensor.matmul(
                                ps[:],
                                xt[:, hc, t * 128:(t + 1) * 128],
                                w[:, hc, :],
                                start=(hc == 0), stop=(hc == NCH - 1))
                        kb = c * 4 + t
                        nc.vector.tensor_copy(
                            v_aug[:, kb, :, 0:HD],
                            ps[:].rearrange("p (h d) -> p h d", h=8))

            # ---------------- attention ----------------
            # Score groups of 4 k-blocks land in ONE [128, 2048] PSUM tile
            # (layout [i, kb-pair, q]: slice i*1024 + u*512) so a single
            # Exp activation covers both heads of the strip -> fewer, larger
            # ACT instructions (ACT is the phase bottleneck).
            with (
                tc.tile_pool(name="ctxp", bufs=1) as ctxp,
                tc.tile_pool(name="apool", bufs=3) as apool,
                tc.tile_pool(name="misc", bufs=2) as misc,
                tc.tile_pool(name="pst", bufs=1, space="PSUM") as pst,
                tc.tile_pool(name="pctx", bufs=1, space="PSUM") as pctx,
            ):
                ctxall = ctxp.tile([128, 4, QR], BF16)
                # prefetch out-projection weights while ACT streams exps
                nc.gpsimd.dma_start(
                    wo_sb[:], wo_s[:].rearrange("(hc p) c -> p hc c", p=128))
                for s in range(4):
                    for qb in range(2):
                        qsl = slice(qb * 512, (qb + 1) * 512)
                        ctxs = [pctx.tile([HD + 1, 512], F32, tag=f"ctx{i}",
                                          name=f"ctx{i}_{s}_{qb}_{_rep}")
                                for i in range(2)]
                        for g0 in range(0, NKB, GKB):
                            kbs = list(range(g0, min(g0 + GKB, NKB)))
                            L = len(kbs)
                            sts = [pst.tile([128, GKB * 512], F32,
                                            tag=f"st{i}",
                                            name=f"st{i}_{s}_{qb}_{g0}_{_rep}")
                                   for i in range(2)]
                            for j, kb in enumerate(kbs):
                                ksl = slice(kb * 128, (kb + 1) * 128)
                                jsl = slice(j * 512, (j + 1) * 512)
                                for i in range(2):
                                    psl = slice(i * 64, (i + 1) * 64)
                                    nc.tensor.matmul(
                                        sts[i][:, jsl],
                                        kTh[psl, s, ksl],
                                        qTh[psl, s, qsl],
                                        start=True, stop=True)
                            As = []
                            for i in range(2):
                                a = apool.tile([128, GKB * 512], BF16,
                                               tag=f"A{i}",
                                               name=f"A{i}_{s}_{qb}_{g0}_{_rep}")
                                nc.scalar.activation(
                                    a[:, 0:L * 512], sts[i][:, 0:L * 512],
                                    mybir.ActivationFunctionType.Exp,
                                    scale=SCALE)
                                As.append(a)
                            for j, kb in enumerate(kbs):
                                jsl = slice(j * 512, (j + 1) * 512)
                                for i in range(2):
                                    nc.tensor.matmul(
                                        ctxs[i][:],
                                        v_aug[:, kb, 2 * s + i, :],
                                        As[i][:, jsl],
                                        start=(kb == 0), stop=(kb == NKB - 1))
                        for i in range(2):
                            rinv = misc.tile([1, 512], F32, tag="rinv")
                            nc.vector.reciprocal(rinv[:], ctxs[i][HD:HD + 1, :])
                            rb = misc.tile([HD, 512], F32, tag="rb")
                            nc.gpsimd.partition_broadcast(rb[:], rinv[:])
                            nc.vector.tensor_mul(
                                ctxall[i * 64:(i + 1) * 64, s, qsl],
                                ctxs[i][0:HD, :], rb[:])
                    nc.sync.dma_start(ctx_own[s * 128:(s + 1) * 128, :],
                                      ctxall[:, s, :])
                    nc.gpsimd.collective_compute(
                        "AllGather", mybir.AluOpType.bypass,
                        ins=[ctx_own[s * 128:(s + 1) * 128, :]],
                        outs=[ctx_gath[s]],
                        replica_groups=[[0, 4], [1, 5], [2, 6], [3, 7]])
                    # pull this strip's gathered pair blocks while later
                    # strips are still computing (DVE queue is mostly idle)
                    for gp in range(2):
                        nc.gpsimd.dma_start(
                            ctxg[:, gp * 4 + s, :],
                            ctx_gath[s, gp * 128:(gp + 1) * 128, :])

            # ---------------- output projection (own 512-col half) ---------
            # ctxg chunk hc = (g', s') = (hc//4, hc%4) was prefetched from
            # ctx_gath[s'][g'*128:(g'+1)*128] as each strip's gather landed.
            with (
                tc.tile_pool(name="osb", bufs=4) as osb,
                tc.tile_pool(name="pwo", bufs=4, space="PSUM") as pwo,
            ):
                for oc in range(OC // 128):
                    for half in range(2):
                        ps = pwo.tile([128, 512], F32, tag="po")
                        hsl = slice(half * 512, (half + 1) * 512)
                        for hc in range(NCH):
                            nc.tensor.matmul(
                                ps[:],
                                wo_sb[:, hc, oc * 128:(oc + 1) * 128],
                                ctxg[:, hc, hsl],
                                start=(hc == 0), stop=(hc == NCH - 1))
                        ot = osb.tile([128, 512], F32, tag="ot")
                        nc.vector.tensor_copy(ot[:], ps[:])
                        nc.sync.dma_start(
                            outT_h[oc * 128:(oc + 1) * 128, hsl], ot[:])

            _pst.close()

    nc.compile()
    return nc
